# revision 8
# baseline (speedup 1.0000x reference)
"""EuclideanVisitEncoder forward: masked-mean embedding bag on 8 NeuronCores.

out[b, :] = sum_l (ids[b,l] != 0) * T[ids[b,l], :] / max(count_b, 1)

Sharding: data-parallel over the batch across 8 cores (25088 padded rows
each); the 6.4 MB table is replicated (stays in each core's DRAM; row 0 is
zeroed host-side so pad ids gather zeros).

The only usable data-dependent addressing primitive on this image is the
qPoolDynamic vector-indirect DMA: ONE offset per partition per
instruction (128 gathered rows), descriptor payload = the dest slice's
per-partition size.  Its issue rate is the bottleneck; measured 857 ns
per instruction when the Pool engine streams an uninterrupted burst, vs
~1.5-2.2 us when ids-loads / DVE ops / output stores are interleaved
every 64 gathers (the previous per-128-row-tile structure).

So the kernel processes SUPERTILES of 512 rows (4 x 128): 4 id-block
DMAs, then a single burst of 256 indirect gathers into one
[128, 256*16] SBUF tile, then one batched DVE mask/count/reduce pass
and 4 output stores.  Pool stays saturated; everything else hides
behind it (DVE work is ~6 us vs ~220 us of gathers per supertile).
"""

import numpy as np

PAD_IDX = 0
NUM_CODES = 100000
DIM = 16
B, L = 200000, 64
N_CORES = 8

TILE_ROWS = 128
TPG = 4                               # 128-row tiles per supertile
SUPER_ROWS = TILE_ROWS * TPG          # 512
B_SHARD = 25088                       # 25000 + pad to multiple of 512
N_SUPER = B_SHARD // SUPER_ROWS       # 49

_PROGRAM_CACHE = {}


def build_program(b_shard=B_SHARD, repeats=1, unroll=7, bufs=4, nq=4):
    from contextlib import ExitStack

    import concourse.tile as tile
    from concourse import bacc, bass, mybir

    n_super = b_shard // SUPER_ROWS
    assert b_shard % SUPER_ROWS == 0 and n_super % unroll == 0
    K = TPG * L                        # gathers per supertile (256)

    nc = bacc.Bacc("TRN2", target_bir_lowering=False, debug=False, num_swdge_queues=nq)
    ids_t = nc.dram_tensor("code_ids", [b_shard, L], mybir.dt.int32, kind="ExternalInput")
    tbl_t = nc.dram_tensor("emb_weight", [NUM_CODES, DIM], mybir.dt.float32, kind="ExternalInput")
    out_t = nc.dram_tensor("out", [b_shard, DIM], mybir.dt.float32, kind="ExternalOutput")

    with ExitStack() as ctx:
        tc = ctx.enter_context(tile.TileContext(nc))
        ids_pool = ctx.enter_context(tc.tile_pool(name="ids", bufs=bufs))
        g_pool = ctx.enter_context(tc.tile_pool(name="g", bufs=bufs))
        s_pool = ctx.enter_context(tc.tile_pool(name="s", bufs=bufs))

        def super_body(row0_expr, qid=0):
            """row0_expr: element row offset of the 512-row supertile.
            qid: SWDGE queue for this supertile's gather burst; rotating it
            across supertiles lets the 64B-descriptor transfers of adjacent
            supertiles run on different queues/DMA engines concurrently."""
            ids_tile = ids_pool.tile([128, K], mybir.dt.int32, tag="ids", name=f"ids{nc.next_id()}")
            for t in range(TPG):
                nc.sync.dma_start(
                    out=ids_tile[:, t * L : (t + 1) * L],
                    in_=ids_t[bass.ds(row0_expr + t * TILE_ROWS, TILE_ROWS), :],
                )
            g = g_pool.tile([128, K * DIM], mybir.dt.float32, tag="g", name=f"g{nc.next_id()}")
            for k in range(K):
                bi = nc.gpsimd.indirect_dma_start(
                    out=g[:, k * DIM : (k + 1) * DIM],
                    out_offset=None,
                    in_=tbl_t[:, :],
                    in_offset=bass.IndirectOffsetOnAxis(ap=ids_tile[:, k : k + 1], axis=0),
                )
                if qid:
                    bi.ins.queue = f"qPoolDynamic{qid}"
            mask = s_pool.tile([128, K], mybir.dt.float32, tag="mask", name=f"m{nc.next_id()}")
            nc.vector.tensor_scalar(mask[:], ids_tile[:], 0, None, op0=mybir.AluOpType.not_equal)
            den = s_pool.tile([128, TPG], mybir.dt.float32, tag="den", name=f"d{nc.next_id()}")
            nc.vector.tensor_reduce(
                den[:],
                mask[:].rearrange("p (t l) -> p t l", t=TPG, l=L),
                axis=mybir.AxisListType.X,
                op=mybir.AluOpType.add,
            )
            nc.vector.tensor_scalar_max(den[:], den[:], 1.0)
            recip = s_pool.tile([128, TPG], mybir.dt.float32, tag="recip", name=f"r{nc.next_id()}")
            nc.vector.reciprocal(recip[:], den[:])
            acc = s_pool.tile([128, TPG * DIM], mybir.dt.float32, tag="acc", name=f"a{nc.next_id()}")
            for t in range(TPG):
                nc.vector.tensor_reduce(
                    acc[:, t * DIM : (t + 1) * DIM],
                    g[:, t * L * DIM : (t + 1) * L * DIM].rearrange(
                        "p (l d) -> p d l", l=L, d=DIM
                    ),
                    axis=mybir.AxisListType.X,
                    op=mybir.AluOpType.add,
                )
            outt = s_pool.tile([128, TPG * DIM], mybir.dt.float32, tag="outt", name=f"o{nc.next_id()}")
            for t in range(TPG):
                nc.vector.tensor_scalar(
                    outt[:, t * DIM : (t + 1) * DIM],
                    acc[:, t * DIM : (t + 1) * DIM],
                    recip[:, t : t + 1],
                    None,
                    op0=mybir.AluOpType.mult,
                )
            for t in range(TPG):
                nc.sync.dma_start(
                    out=out_t[bass.ds(row0_expr + t * TILE_ROWS, TILE_ROWS), :],
                    in_=outt[:, t * DIM : (t + 1) * DIM],
                )

        if repeats == 1:
            with tc.For_i(0, n_super // unroll, 1) as i:
                for u in range(unroll):
                    super_body(i * (SUPER_ROWS * unroll) + u * SUPER_ROWS, qid=u % nq)
        else:
            # timing variant: repeat the whole shard computation
            with tc.For_i(0, repeats, 1) as _r:
                with tc.For_i(0, n_super // unroll, 1) as i:
                    for u in range(unroll):
                        super_body(i * (SUPER_ROWS * unroll) + u * SUPER_ROWS, qid=u % nq)

    nc.compile()
    return nc


def _get_program():
    key = (B_SHARD, 1)
    if key not in _PROGRAM_CACHE:
        _PROGRAM_CACHE[key] = build_program()
    return _PROGRAM_CACHE[key]


def make_in_maps(code_ids: np.ndarray, emb_weight: np.ndarray):
    code_ids = np.ascontiguousarray(np.asarray(code_ids), dtype=np.int32)
    emb_weight = np.ascontiguousarray(np.asarray(emb_weight), dtype=np.float32)
    tbl = emb_weight.copy()
    tbl[PAD_IDX, :] = 0.0
    b_total = N_CORES * B_SHARD
    ids_pad = np.zeros((b_total, L), dtype=np.int32)
    ids_pad[: code_ids.shape[0], :] = code_ids
    return [
        {
            "code_ids": ids_pad[i * B_SHARD : (i + 1) * B_SHARD],
            "emb_weight": tbl,
        }
        for i in range(N_CORES)
    ]


def kernel(code_ids: np.ndarray, emb_weight: np.ndarray, **kwargs) -> np.ndarray:
    from concourse import bass_utils

    nc = _get_program()
    in_maps = make_in_maps(code_ids, emb_weight)
    res = bass_utils.run_bass_kernel_spmd(nc, in_maps, core_ids=list(range(N_CORES)))
    out = np.concatenate([res.results[i]["out"] for i in range(N_CORES)], axis=0)
    return out[: np.asarray(code_ids).shape[0]]


if __name__ == "__main__":
    rng = np.random.default_rng(0)
    ids = rng.integers(0, NUM_CODES, size=(B, L)).astype(np.int32)
    w = rng.standard_normal((NUM_CODES, DIM)).astype(np.float32)
    o = kernel(code_ids=ids, emb_weight=w)
    print(o.shape, o.dtype, o[:2, :4])


# revision 9
# speedup vs baseline: 1.0664x; 1.0664x over previous
"""EuclideanVisitEncoder forward: masked-mean embedding bag on 8 NeuronCores.

out[b, :] = sum_l (ids[b,l] != 0) * T[ids[b,l], :] / max(count_b, 1)

Sharding: data-parallel over the batch across 8 cores (25088 padded rows
each); the 6.4 MB table is replicated (stays in each core's DRAM; row 0 is
zeroed host-side so pad ids gather zeros).

The only usable data-dependent addressing primitive on this image is the
qPoolDynamic vector-indirect DMA: ONE offset per partition per
instruction (128 gathered rows), descriptor payload = the dest slice's
per-partition size.  Its issue rate is the bottleneck; measured 857 ns
per instruction when the Pool engine streams an uninterrupted burst, vs
~1.5-2.2 us when ids-loads / DVE ops / output stores are interleaved
every 64 gathers (the previous per-128-row-tile structure).

So the kernel processes SUPERTILES of 512 rows (4 x 128): 4 id-block
DMAs, then a single burst of 256 indirect gathers into one
[128, 256*16] SBUF tile, then one batched DVE mask/count/reduce pass
and 4 output stores.  Pool stays saturated; everything else hides
behind it (DVE work is ~6 us vs ~220 us of gathers per supertile).
"""

import numpy as np

PAD_IDX = 0
NUM_CODES = 100000
DIM = 16
B, L = 200000, 64
N_CORES = 8

TILE_ROWS = 128
TPG = 4                               # 128-row tiles per supertile
SUPER_ROWS = TILE_ROWS * TPG          # 512
B_SHARD = 25088                       # 25000 + pad to multiple of 512
N_SUPER = B_SHARD // SUPER_ROWS       # 49

_PROGRAM_CACHE = {}


def build_program(b_shard=B_SHARD, repeats=1, unroll=7, bufs=6, nq=4):
    from contextlib import ExitStack

    import concourse.tile as tile
    from concourse import bacc, bass, mybir

    n_super = b_shard // SUPER_ROWS
    assert b_shard % SUPER_ROWS == 0 and n_super % unroll == 0
    K = TPG * L                        # gathers per supertile (256)

    nc = bacc.Bacc("TRN2", target_bir_lowering=False, debug=False, num_swdge_queues=nq)
    ids_t = nc.dram_tensor("code_ids", [b_shard, L], mybir.dt.int32, kind="ExternalInput")
    tbl_t = nc.dram_tensor("emb_weight", [NUM_CODES, DIM], mybir.dt.float32, kind="ExternalInput")
    out_t = nc.dram_tensor("out", [b_shard, DIM], mybir.dt.float32, kind="ExternalOutput")

    with ExitStack() as ctx:
        tc = ctx.enter_context(tile.TileContext(nc))
        ids_pool = ctx.enter_context(tc.tile_pool(name="ids", bufs=bufs))
        g_pool = ctx.enter_context(tc.tile_pool(name="g", bufs=bufs))
        s_pool = ctx.enter_context(tc.tile_pool(name="s", bufs=bufs))

        def super_body(row0_expr, qid=0):
            """row0_expr: element row offset of the 512-row supertile.
            qid: SWDGE queue for this supertile's gather burst; rotating it
            across supertiles lets the 64B-descriptor transfers of adjacent
            supertiles run on different queues/DMA engines concurrently."""
            ids_tile = ids_pool.tile([128, K], mybir.dt.int32, tag="ids", name=f"ids{nc.next_id()}")
            for t in range(TPG):
                nc.sync.dma_start(
                    out=ids_tile[:, t * L : (t + 1) * L],
                    in_=ids_t[bass.ds(row0_expr + t * TILE_ROWS, TILE_ROWS), :],
                )
            g = g_pool.tile([128, K * DIM], mybir.dt.float32, tag="g", name=f"g{nc.next_id()}")
            for k in range(K):
                bi = nc.gpsimd.indirect_dma_start(
                    out=g[:, k * DIM : (k + 1) * DIM],
                    out_offset=None,
                    in_=tbl_t[:, :],
                    in_offset=bass.IndirectOffsetOnAxis(ap=ids_tile[:, k : k + 1], axis=0),
                )
                if qid:
                    bi.ins.queue = f"qPoolDynamic{qid}"
            mask = s_pool.tile([128, K], mybir.dt.float32, tag="mask", name=f"m{nc.next_id()}")
            nc.vector.tensor_scalar(mask[:], ids_tile[:], 0, None, op0=mybir.AluOpType.not_equal)
            den = s_pool.tile([128, TPG], mybir.dt.float32, tag="den", name=f"d{nc.next_id()}")
            nc.vector.tensor_reduce(
                den[:],
                mask[:].rearrange("p (t l) -> p t l", t=TPG, l=L),
                axis=mybir.AxisListType.X,
                op=mybir.AluOpType.add,
            )
            nc.vector.tensor_scalar_max(den[:], den[:], 1.0)
            recip = s_pool.tile([128, TPG], mybir.dt.float32, tag="recip", name=f"r{nc.next_id()}")
            nc.vector.reciprocal(recip[:], den[:])
            acc = s_pool.tile([128, TPG * DIM], mybir.dt.float32, tag="acc", name=f"a{nc.next_id()}")
            for t in range(TPG):
                nc.vector.tensor_reduce(
                    acc[:, t * DIM : (t + 1) * DIM],
                    g[:, t * L * DIM : (t + 1) * L * DIM].rearrange(
                        "p (l d) -> p d l", l=L, d=DIM
                    ),
                    axis=mybir.AxisListType.X,
                    op=mybir.AluOpType.add,
                )
            outt = s_pool.tile([128, TPG * DIM], mybir.dt.float32, tag="outt", name=f"o{nc.next_id()}")
            for t in range(TPG):
                nc.vector.tensor_scalar(
                    outt[:, t * DIM : (t + 1) * DIM],
                    acc[:, t * DIM : (t + 1) * DIM],
                    recip[:, t : t + 1],
                    None,
                    op0=mybir.AluOpType.mult,
                )
            for t in range(TPG):
                nc.sync.dma_start(
                    out=out_t[bass.ds(row0_expr + t * TILE_ROWS, TILE_ROWS), :],
                    in_=outt[:, t * DIM : (t + 1) * DIM],
                )

        if repeats == 1:
            with tc.For_i(0, n_super // unroll, 1) as i:
                for u in range(unroll):
                    super_body(i * (SUPER_ROWS * unroll) + u * SUPER_ROWS, qid=u % nq)
        else:
            # timing variant: repeat the whole shard computation
            with tc.For_i(0, repeats, 1) as _r:
                with tc.For_i(0, n_super // unroll, 1) as i:
                    for u in range(unroll):
                        super_body(i * (SUPER_ROWS * unroll) + u * SUPER_ROWS, qid=u % nq)

    nc.compile()
    return nc


def _get_program():
    key = (B_SHARD, 1)
    if key not in _PROGRAM_CACHE:
        _PROGRAM_CACHE[key] = build_program()
    return _PROGRAM_CACHE[key]


def make_in_maps(code_ids: np.ndarray, emb_weight: np.ndarray):
    code_ids = np.ascontiguousarray(np.asarray(code_ids), dtype=np.int32)
    emb_weight = np.ascontiguousarray(np.asarray(emb_weight), dtype=np.float32)
    tbl = emb_weight.copy()
    tbl[PAD_IDX, :] = 0.0
    b_total = N_CORES * B_SHARD
    ids_pad = np.zeros((b_total, L), dtype=np.int32)
    ids_pad[: code_ids.shape[0], :] = code_ids
    return [
        {
            "code_ids": ids_pad[i * B_SHARD : (i + 1) * B_SHARD],
            "emb_weight": tbl,
        }
        for i in range(N_CORES)
    ]


def kernel(code_ids: np.ndarray, emb_weight: np.ndarray, **kwargs) -> np.ndarray:
    from concourse import bass_utils

    nc = _get_program()
    in_maps = make_in_maps(code_ids, emb_weight)
    res = bass_utils.run_bass_kernel_spmd(nc, in_maps, core_ids=list(range(N_CORES)))
    out = np.concatenate([res.results[i]["out"] for i in range(N_CORES)], axis=0)
    return out[: np.asarray(code_ids).shape[0]]


if __name__ == "__main__":
    rng = np.random.default_rng(0)
    ids = rng.integers(0, NUM_CODES, size=(B, L)).astype(np.int32)
    w = rng.standard_normal((NUM_CODES, DIM)).astype(np.float32)
    o = kernel(code_ids=ids, emb_weight=w)
    print(o.shape, o.dtype, o[:2, :4])
